# revision 14
# baseline (speedup 1.0000x reference)
# Trainium2 Bass kernel for GQA attention block (RMSNorm -> QKV -> RoPE ->
# causal attention -> output projection), tensor-parallel over heads on 8
# NeuronCores.
#
# Self-contained: hardcodes all shapes; host-side prep shards/permutes the
# inputs, the device program is identical SPMD on cores 0-7, and the host
# concatenates the per-core output column blocks.
import math

import numpy as np
import ml_dtypes

import concourse.bass as bass
import concourse.mybir as mybir
import concourse.tile as tile
from concourse import bacc
from concourse.bass_utils import run_bass_kernel_spmd

BF = ml_dtypes.bfloat16

SEQ = 2048
DIM = 4096
HD = 128
N_HEADS = 32
N_KV = 8
NCORES = 8
QH = N_HEADS // NCORES          # 4 q-heads per core
DQ = QH * HD                    # 512
KT = DIM // 128                 # 32 contraction tiles
SB = SEQ // 128                 # 16 seq blocks
QC = SEQ // 512                 # 4 seq chunks
ROPE_THETA = 50000.0
EPS = 1e-5
SCALE = 1.0 / math.sqrt(HD)

f32 = mybir.dt.float32
bf16 = mybir.dt.bfloat16

# stash of the last run's BassKernelResults (for test.py to read timing)
LAST_RESULT = None


def host_prep(hidden, norm_w, wq, wk, wv, wo):
    """Build the 8 per-core input maps (numpy, host-side)."""
    hidden = np.asarray(hidden, dtype=np.float32)
    norm_w = np.asarray(norm_w, dtype=np.float32)
    wq = np.asarray(wq, dtype=np.float32)
    wk = np.asarray(wk, dtype=np.float32)
    wv = np.asarray(wv, dtype=np.float32)
    wo = np.asarray(wo, dtype=np.float32)

    # rope pair permutation within a head: evens (2i) first, odds (2i+1) last
    perm = np.concatenate([np.arange(0, HD, 2), np.arange(1, HD, 2)])

    inv_freq = 1.0 / (ROPE_THETA ** (np.arange(0, HD, 2, dtype=np.float64) / HD))
    t = np.arange(SEQ, dtype=np.float64)
    ang = t[None, :] * inv_freq[:, None]            # [64, SEQ]
    cos64 = np.cos(ang)
    sin64 = np.sin(ang)
    cosb = np.concatenate([cos64, cos64], axis=0).astype(np.float32)       # [128,SEQ]
    # +sin on top half, -sin on bottom half (pre-swapped sign trick)
    sinb_pre = np.concatenate([sin64, -sin64], axis=0).astype(np.float32)  # [128,SEQ]

    maskdiag = np.triu(np.ones((128, 128), np.float32)).astype(BF)  # 1 iff p<=f
    ones128 = np.ones((128, 128), BF)
    ident = np.eye(128, dtype=BF)

    hT = np.ascontiguousarray(hidden.T).astype(BF)  # [DIM, SEQ]

    wn = norm_w[None, :]
    maps = []
    for c in range(NCORES):
        wq_c = (wq[c * DQ:(c + 1) * DQ] * wn).reshape(QH, HD, DIM)[:, perm, :]
        wq_c = wq_c.reshape(DQ, DIM)
        wk_c = (wk[c * HD:(c + 1) * HD] * wn)[perm, :]
        wv_c = wv[c * HD:(c + 1) * HD] * wn
        wo_c = wo[c * DQ:(c + 1) * DQ]              # [DQ(j), DIM(i)]
        # per-head staggered AllGather concatenates rank-major per head:
        # gathered group h rows = [global head 4*r+h for rank r] * 128.
        # reorder woT rows (i) to match.
        order = np.concatenate(
            [np.arange((4 * r + h) * HD, (4 * r + h + 1) * HD)
             for h in range(QH) for r in range(NCORES)])
        maps.append({
            "hT": hT,
            "wqT": np.ascontiguousarray(wq_c.T).astype(BF),   # [DIM, DQ]
            "wkT": np.ascontiguousarray(wk_c.T).astype(BF),   # [DIM, HD]
            "wvT": np.ascontiguousarray(wv_c.T).astype(BF),   # [DIM, HD]
            "woT": np.ascontiguousarray(wo_c.T[order]).astype(BF),  # [DIM, DQ]
            "cosb": cosb,
            "sinb_pre": sinb_pre,
            "maskdiag": np.ascontiguousarray(maskdiag),
            "ones128": ones128,
            "ident": ident,
        })
    return maps


def build_body(nc, tc, ins, out_ap):
    """Emit the Tile program. ins: dict name -> AP. out_ap: [SEQ, DQ] f32."""
    HALF = SEQ // 2              # 1024

    with tc.tile_pool(name="consts", bufs=1) as consts:
        mask_sb = consts.tile([128, 128], bf16, name="mask_sb")
        nc.sync.dma_start(mask_sb[:], ins["maskdiag"])
        ones_sb = consts.tile([128, 128], bf16, name="ones_sb")
        nc.sync.dma_start(ones_sb[:], ins["ones128"][:, 0:128])
        ident_sb = consts.tile([128, 128], bf16, name="ident_sb")
        nc.sync.dma_start(ident_sb[:], ins["ident"])
        # explicit bias tiles (raw builds have no preamble const APs)
        eps_f32 = consts.tile([128, 1], f32, name="eps_f32")
        nc.vector.memset(eps_f32[:], EPS)
        zero_f32 = consts.tile([128, 1], f32, name="zero_f32")
        nc.vector.memset(zero_f32[:], 0.0)
        zero_bf16 = consts.tile([128, 1], bf16, name="zero_bf16")
        nc.vector.memset(zero_bf16[:], 0.0)

        with tc.tile_pool(name="qkvout", bufs=1) as qkvout, \
             tc.tile_pool(name="dram", bufs=1, space="DRAM") as dramp:
            QT = [qkvout.tile([128, SEQ], bf16, name=f"qt{b}") for b in range(QH)]
            KTile = qkvout.tile([128, SEQ], bf16, name="ktile")
            Vn = qkvout.tile([128, SB, 128], bf16, name="vn")
            ag_in = [dramp.tile([HD, SEQ], bf16, name=f"ag_in{h}",
                                tag=f"agin{h}", bufs=1)
                     for h in range(QH)]
            ag_out = [dramp.tile([NCORES * HD, SEQ], bf16, name=f"ag_out{h}",
                                 tag=f"agout{h}", bufs=1,
                                 addr_space="Shared") for h in range(QH)]

            # ---------------- phase 1: stats + QKV + rope + V ----------------
            with tc.tile_pool(name="ht", bufs=1) as htp, \
                 tc.tile_pool(name="wts", bufs=1) as wtp, \
                 tc.tile_pool(name="trig", bufs=1) as trig, \
                 tc.tile_pool(name="p1tmp", bufs=1) as tmp, \
                 tc.tile_pool(name="psr", bufs=1, space="PSUM") as psr, \
                 tc.tile_pool(name="psqkv", bufs=4, space="PSUM") as psqkv, \
                 tc.tile_pool(name="psvt", bufs=2, space="PSUM") as psvt:

                cosr = trig.tile([128, SEQ], f32, name="cosr")
                nc.sync.dma_start(cosr[:], ins["cosb"])
                sinr = trig.tile([128, SEQ], f32, name="sinr")
                nc.sync.dma_start(sinr[:], ins["sinb_pre"])
                Rr = trig.tile([128, SEQ], f32, name="rr")

                wq_sb = wtp.tile([128, KT, DQ], bf16, name="wq_sb")
                wqT_ap = ins["wqT"].rearrange("(kt p) d -> p kt d", p=128)
                for kt in range(KT):
                    nc.sync.dma_start(wq_sb[:, kt, :], wqT_ap[:, kt, :])
                wkT_ap = ins["wkT"].rearrange("(kt p) d -> p kt d", p=128)
                wvT_ap = ins["wvT"].rearrange("(kt p) d -> p kt d", p=128)
                hT_ap = ins["hT"].rearrange("(kt p) s -> p kt s", p=128)

                for half in range(2):
                    h0 = half * HALF
                    hts = []
                    for kt in range(KT):
                        t = htp.tile([128, HALF], bf16, tag="ht", bufs=33,
                                     name=f"ht_{half}_{kt}")
                        nc.sync.dma_start(t[:], hT_ap[:, kt, h0:h0 + HALF])
                        hts.append(t)

                    # --- rms stats: sumsq over dim via ACT square + ones-matmul
                    ps_r = psr.tile([128, HALF], f32, tag="psr", name="ps_r")
                    for kt in range(KT):
                        sq = tmp.tile([128, HALF], bf16, tag="sq", bufs=2,
                                      name=f"sq_{half}_{kt}")
                        nc.scalar.activation(
                            sq[:], hts[kt][:],
                            mybir.ActivationFunctionType.Square,
                            bias=zero_bf16[:])
                        for c2 in range(2):
                            nc.tensor.matmul(
                                ps_r[:, c2 * 512:(c2 + 1) * 512],
                                lhsT=ones_sb[:],
                                rhs=sq[:, c2 * 512:(c2 + 1) * 512],
                                start=(kt == 0), stop=(kt == KT - 1))
                    srt = tmp.tile([128, HALF], f32, tag="srt", bufs=1,
                                   name=f"srt_{half}")
                    # sqrt(sumsq/DIM + EPS)
                    nc.scalar.activation(
                        srt[:], ps_r[:], mybir.ActivationFunctionType.Sqrt,
                        bias=eps_f32[:], scale=1.0 / DIM)
                    nc.vector.reciprocal(Rr[:, h0:h0 + HALF], srt[:])
                    nc.vector.tensor_mul(
                        out=cosr[:, h0:h0 + HALF], in0=cosr[:, h0:h0 + HALF],
                        in1=Rr[:, h0:h0 + HALF])
                    nc.vector.tensor_mul(
                        out=sinr[:, h0:h0 + HALF], in0=sinr[:, h0:h0 + HALF],
                        in1=Rr[:, h0:h0 + HALF])

                    # --- QKV projections: blocks 0..3 = q heads, 4 = k, 5 = v
                    # K/V first so attention S-matmuls can be hoisted by the
                    # scheduler as soon as each q head completes
                    for b in (4, 5, 0, 1, 2, 3):
                        for ch in range(HALF // 512):
                            s0 = h0 + ch * 512
                            sl = slice(ch * 512, (ch + 1) * 512)
                            if b == 4:
                                wtile = [tmp.tile([128, 128], bf16, tag="wkv",
                                                  bufs=8, name=f"wk_{half}_{ch}_{kt}")
                                         for kt in range(KT)]
                                for kt in range(KT):
                                    nc.sync.dma_start(wtile[kt][:], wkT_ap[:, kt, :])
                            elif b == 5:
                                wtile = [tmp.tile([128, 128], bf16, tag="wkv",
                                                  bufs=8, name=f"wv_{half}_{ch}_{kt}")
                                         for kt in range(KT)]
                                for kt in range(KT):
                                    nc.sync.dma_start(wtile[kt][:], wvT_ap[:, kt, :])
                            ps = psqkv.tile([128, 512], f32, tag="qkv",
                                            name=f"ps_{half}_{b}_{ch}")
                            for kt in range(KT):
                                if b < 4:
                                    lhsT = wq_sb[:, kt, b * 128:(b + 1) * 128]
                                else:
                                    lhsT = wtile[kt][:]
                                nc.tensor.matmul(
                                    ps[:], lhsT=lhsT, rhs=hts[kt][:, sl],
                                    start=(kt == 0), stop=(kt == KT - 1))
                            if b == 5:
                                # V: scale by r, transpose 128-blocks to natural
                                vsc = tmp.tile([128, 512], bf16, tag="vsc",
                                               bufs=2, name=f"vsc_{half}_{ch}")
                                nc.vector.tensor_mul(
                                    out=vsc[:], in0=ps[:],
                                    in1=Rr[:, s0:s0 + 512])
                                for j in range(4):
                                    pvt = psvt.tile([128, 128], bf16, tag="vt",
                                                    name=f"pvt_{half}_{ch}_{j}")
                                    nc.tensor.transpose(
                                        pvt[:], vsc[:, j * 128:(j + 1) * 128],
                                        ident_sb[:])
                                    nc.vector.tensor_copy(
                                        out=Vn[:, s0 // 128 + j, :], in_=pvt[:])
                            else:
                                # rope: out = x*cosr + swap(x*sinr_pre)
                                dst = QT[b] if b < QH else KTile
                                m1 = tmp.tile([128, 512], f32, tag="m1", bufs=2,
                                              name=f"m1_{half}_{b}_{ch}")
                                nc.vector.tensor_mul(
                                    out=m1[:], in0=ps[:], in1=cosr[:, s0:s0 + 512])
                                m2p = tmp.tile([128, 512], f32, tag="m2p", bufs=2,
                                               name=f"m2p_{half}_{b}_{ch}")
                                nc.vector.tensor_mul(
                                    out=m2p[:], in0=ps[:], in1=sinr[:, s0:s0 + 512])
                                m2 = tmp.tile([128, 512], f32, tag="m2", bufs=2,
                                              name=f"m2_{half}_{b}_{ch}")
                                nc.sync.dma_start(m2[0:64, :], m2p[64:128, :])
                                nc.sync.dma_start(m2[64:128, :], m2p[0:64, :])
                                nc.vector.tensor_add(
                                    out=dst[:, s0:s0 + 512], in0=m1[:], in1=m2[:])

            # ---------------- phase 2: causal attention (transposed scores) --
            with tc.tile_pool(name="p2", bufs=1) as p2, \
                 tc.tile_pool(name="pss", bufs=3, space="PSUM") as pss, \
                 tc.tile_pool(name="psav", bufs=2, space="PSUM") as psav, \
                 tc.tile_pool(name="psden", bufs=2, space="PSUM") as psden:
                for h in range(QH):
                    for qc in range(QC):
                        av = psav.tile([128, 512], f32, tag="av",
                                       name=f"av_{h}_{qc}")
                        den = psden.tile([128, 512], f32, tag="den",
                                         name=f"den_{h}_{qc}")
                        nkb = 4 * qc + 4
                        for kb in range(nkb):
                            ss = pss.tile([128, 512], f32, tag="s",
                                          name=f"ss_{h}_{qc}_{kb}")
                            nc.tensor.matmul(
                                ss[:], lhsT=KTile[:, kb * 128:(kb + 1) * 128],
                                rhs=QT[h][:, qc * 512:(qc + 1) * 512],
                                start=True, stop=True)
                            es = p2.tile([128, 512], bf16, tag="es", bufs=18,
                                         name=f"es_{h}_{qc}_{kb}")
                            j = kb - 4 * qc
                            if j < 0:
                                nc.scalar.activation(
                                    es[:], ss[:],
                                    mybir.ActivationFunctionType.Exp,
                                    bias=zero_f32[:], scale=SCALE)
                            else:
                                if j > 0:
                                    nc.vector.memset(es[:, 0:j * 128], 0.0)
                                nc.scalar.activation(
                                    es[:, j * 128:512], ss[:, j * 128:512],
                                    mybir.ActivationFunctionType.Exp,
                                    bias=zero_f32[:], scale=SCALE)
                                nc.vector.tensor_mul(
                                    out=es[:, j * 128:(j + 1) * 128],
                                    in0=es[:, j * 128:(j + 1) * 128],
                                    in1=mask_sb[:])
                            nc.tensor.matmul(
                                av[:], lhsT=Vn[:, kb, :], rhs=es[:],
                                start=(kb == 0), stop=(kb == nkb - 1))
                            nc.tensor.matmul(
                                den[:], lhsT=ones_sb[:], rhs=es[:],
                                start=(kb == 0), stop=(kb == nkb - 1))
                        dinv = p2.tile([128, 512], f32, tag="dinv", bufs=2,
                                       name=f"dinv_{h}_{qc}")
                        nc.vector.reciprocal(dinv[:], den[:])
                        ao = p2.tile([128, 512], bf16, tag="ao", bufs=2,
                                     name=f"ao_{h}_{qc}")
                        nc.vector.tensor_mul(out=ao[:], in0=av[:], in1=dinv[:])
                        nc.sync.dma_start(
                            ag_in[h][:, qc * 512:(qc + 1) * 512], ao[:])

                    # gather this head across ranks while the next head's
                    # attention keeps the engines busy
                    nc.gpsimd.collective_compute(
                        "AllGather", mybir.AluOpType.bypass,
                        replica_groups=[list(range(NCORES))],
                        ins=[ag_in[h][:].opt()], outs=[ag_out[h][:].opt()])

        # ---------------- phase 3: output projection ----------------
        # i-tiles grouped by gather (8 tiles per per-head AllGather); the wo
        # accumulation walks groups in gather-completion order so the PE only
        # waits on the last AllGather, and only if it has run out of work.
        with tc.tile_pool(name="attn", bufs=1) as ap_, \
             tc.tile_pool(name="wop", bufs=1) as wop, \
             tc.tile_pool(name="otmp", bufs=1) as otmp, \
             tc.tile_pool(name="pso", bufs=8, space="PSUM") as pso:
            wo_sb = wop.tile([128, KT, DQ], bf16, name="wo_sb")
            woT_ap = ins["woT"].rearrange("(it p) j -> p it j", p=128)
            for it in range(KT):
                nc.sync.dma_start(wo_sb[:, it, :], woT_ap[:, it, :])
            atiles = []   # [g][it8]
            for g in range(QH):
                ag_ap = ag_out[g].rearrange("(it p) s -> p it s", p=128)
                gt = []
                for it8 in range(NCORES):
                    at = ap_.tile([128, SEQ], bf16, tag="attn", bufs=32,
                                  name=f"attn_{g}_{it8}")
                    nc.sync.dma_start(at[:], ag_ap[:, it8, :])
                    gt.append(at)
                atiles.append(gt)
            NPASS = 2
            PSB = SB // NPASS     # 8 s-blocks per pass
            for p_ in range(NPASS):
                pos = [pso.tile([128, DQ], f32, tag="o",
                                name=f"po_{p_}_{i}") for i in range(PSB)]
                for g in range(QH):
                    for it8 in range(NCORES):
                        for i in range(PSB):
                            sb = p_ * PSB + i
                            nc.tensor.matmul(
                                pos[i][:],
                                lhsT=atiles[g][it8][:, sb * 128:(sb + 1) * 128],
                                rhs=wo_sb[:, g * NCORES + it8, :],
                                start=(g == 0 and it8 == 0),
                                stop=(g == QH - 1 and it8 == NCORES - 1))
                for i in range(PSB):
                    sb = p_ * PSB + i
                    ot = otmp.tile([128, DQ], f32, tag="ot", bufs=4,
                                   name=f"ot_{sb}")
                    nc.scalar.copy(ot[:], pos[i][:])
                    nc.sync.dma_start(out_ap[sb * 128:(sb + 1) * 128, :],
                                      ot[:])


def build_program(reps=1):
    nc = bacc.Bacc("TRN2", target_bir_lowering=False, debug=False,
                   num_devices=NCORES)
    in_specs = {
        "hT": ([DIM, SEQ], bf16),
        "wqT": ([DIM, DQ], bf16),
        "wkT": ([DIM, HD], bf16),
        "wvT": ([DIM, HD], bf16),
        "woT": ([DIM, DQ], bf16),
        "cosb": ([128, SEQ], f32),
        "sinb_pre": ([128, SEQ], f32),
        "maskdiag": ([128, 128], bf16),
        # width encodes reps so differently-unrolled builds can't alias in
        # the jit/AOT compile cache (keys include input avals)
        "ones128": ([128, 128 + (reps - 1)], bf16),
        "ident": ([128, 128], bf16),
    }
    ins = {}
    for name, (shape, dt) in in_specs.items():
        ins[name] = nc.dram_tensor(name, shape, dt, kind="ExternalInput").ap()
    out_t = nc.dram_tensor("out", [SEQ, DQ], f32, kind="ExternalOutput")
    with tile.TileContext(nc) as tc:
        for _ in range(reps):
            build_body(nc, tc, ins, out_t.ap())
    nc.compile()
    return nc


def kernel(**inputs):
    global LAST_RESULT
    maps = host_prep(**inputs)
    nc = build_program()
    res = run_bass_kernel_spmd(nc, maps, core_ids=list(range(NCORES)))
    LAST_RESULT = res
    out = np.concatenate([res.results[c]["out"] for c in range(NCORES)], axis=1)
    return out.astype(np.float32)


# revision 16
# speedup vs baseline: 1.1216x; 1.1216x over previous
# Trainium2 Bass kernel for GQA attention block (RMSNorm -> QKV -> RoPE ->
# causal attention -> output projection), tensor-parallel over heads on 8
# NeuronCores.
#
# Self-contained: hardcodes all shapes; host-side prep shards/permutes the
# inputs, the device program is identical SPMD on cores 0-7, and the host
# concatenates the per-core output column blocks.
import math

import numpy as np
import ml_dtypes

import concourse.bass as bass
import concourse.mybir as mybir
import concourse.tile as tile
from concourse import bacc
from concourse.bass_utils import run_bass_kernel_spmd

BF = ml_dtypes.bfloat16

SEQ = 2048
DIM = 4096
HD = 128
N_HEADS = 32
N_KV = 8
NCORES = 8
QH = N_HEADS // NCORES          # 4 q-heads per core
DQ = QH * HD                    # 512
KT = DIM // 128                 # 32 contraction tiles
SB = SEQ // 128                 # 16 seq blocks
QC = SEQ // 512                 # 4 seq chunks
ROPE_THETA = 50000.0
EPS = 1e-5
SCALE = 1.0 / math.sqrt(HD)

f32 = mybir.dt.float32
bf16 = mybir.dt.bfloat16

# stash of the last run's BassKernelResults (for test.py to read timing)
LAST_RESULT = None


def host_prep(hidden, norm_w, wq, wk, wv, wo):
    """Build the 8 per-core input maps (numpy, host-side)."""
    hidden = np.asarray(hidden, dtype=np.float32)
    norm_w = np.asarray(norm_w, dtype=np.float32)
    wq = np.asarray(wq, dtype=np.float32)
    wk = np.asarray(wk, dtype=np.float32)
    wv = np.asarray(wv, dtype=np.float32)
    wo = np.asarray(wo, dtype=np.float32)

    # rope pair permutation within a head: evens (2i) first, odds (2i+1) last
    perm = np.concatenate([np.arange(0, HD, 2), np.arange(1, HD, 2)])

    inv_freq = 1.0 / (ROPE_THETA ** (np.arange(0, HD, 2, dtype=np.float64) / HD))
    t = np.arange(SEQ, dtype=np.float64)
    ang = t[None, :] * inv_freq[:, None]            # [64, SEQ]
    cos64 = np.cos(ang)
    sin64 = np.sin(ang)
    cosb = np.concatenate([cos64, cos64], axis=0).astype(np.float32)       # [128,SEQ]
    # +sin on top half, -sin on bottom half (pre-swapped sign trick)
    sinb_pre = np.concatenate([sin64, -sin64], axis=0).astype(np.float32)  # [128,SEQ]

    maskdiag = np.triu(np.ones((128, 128), np.float32)).astype(BF)  # 1 iff p<=f
    ones128 = np.ones((128, 128), BF)
    ident = np.eye(128, dtype=BF)

    hT = np.ascontiguousarray(hidden.T).astype(BF)  # [DIM, SEQ]

    wn = norm_w[None, :]
    maps = []
    for c in range(NCORES):
        wq_c = (wq[c * DQ:(c + 1) * DQ] * wn).reshape(QH, HD, DIM)[:, perm, :]
        wq_c = wq_c.reshape(DQ, DIM)
        wk_c = (wk[c * HD:(c + 1) * HD] * wn)[perm, :]
        wv_c = wv[c * HD:(c + 1) * HD] * wn
        wo_c = wo[c * DQ:(c + 1) * DQ]              # [DQ(j), DIM(i)]
        # per-head staggered AllGather concatenates rank-major per head:
        # gathered group h rows = [global head 4*r+h for rank r] * 128.
        # reorder woT rows (i) to match.
        order = np.concatenate(
            [np.arange((4 * r + h) * HD, (4 * r + h + 1) * HD)
             for h in range(QH) for r in range(NCORES)])
        maps.append({
            "hT": hT,
            "wqT": np.ascontiguousarray(wq_c.T).astype(BF),   # [DIM, DQ]
            "wkT": np.ascontiguousarray(wk_c.T).astype(BF),   # [DIM, HD]
            "wvT": np.ascontiguousarray(wv_c.T).astype(BF),   # [DIM, HD]
            "woT": np.ascontiguousarray(wo_c.T[order]).astype(BF),  # [DIM, DQ]
            "cosb": cosb,
            "sinb_pre": sinb_pre,
            "maskdiag": np.ascontiguousarray(maskdiag),
            "ones128": ones128,
            "ident": ident,
        })
    return maps


def build_body(nc, tc, ins, out_ap):
    """Emit the Tile program. ins: dict name -> AP. out_ap: [SEQ, DQ] f32."""
    HALF = SEQ // 2              # 1024

    with tc.tile_pool(name="consts", bufs=1) as consts:
        mask_sb = consts.tile([128, 128], bf16, name="mask_sb")
        nc.sync.dma_start(mask_sb[:], ins["maskdiag"])
        ones_sb = consts.tile([128, 128], bf16, name="ones_sb")
        nc.sync.dma_start(ones_sb[:], ins["ones128"][:, 0:128])
        ident_sb = consts.tile([128, 128], bf16, name="ident_sb")
        nc.sync.dma_start(ident_sb[:], ins["ident"])
        # explicit bias tiles (raw builds have no preamble const APs)
        eps_f32 = consts.tile([128, 1], f32, name="eps_f32")
        nc.vector.memset(eps_f32[:], EPS)
        zero_f32 = consts.tile([128, 1], f32, name="zero_f32")
        nc.vector.memset(zero_f32[:], 0.0)
        zero_bf16 = consts.tile([128, 1], bf16, name="zero_bf16")
        nc.vector.memset(zero_bf16[:], 0.0)

        with tc.tile_pool(name="qkvout", bufs=1) as qkvout, \
             tc.tile_pool(name="dram", bufs=1, space="DRAM") as dramp:
            QT = [qkvout.tile([128, SEQ], bf16, name=f"qt{b}") for b in range(QH)]
            KTile = qkvout.tile([128, SEQ], bf16, name="ktile")
            Vn = qkvout.tile([128, SB, 128], bf16, name="vn")
            ag_in = [dramp.tile([HD, SEQ], bf16, name=f"ag_in{h}",
                                tag=f"agin{h}", bufs=1)
                     for h in range(QH)]
            ag_out = [dramp.tile([NCORES * HD, SEQ], bf16, name=f"ag_out{h}",
                                 tag=f"agout{h}", bufs=1,
                                 addr_space="Shared") for h in range(QH)]

            # -------- phase 1+2: stats + QKV + rope + V + attention ----------
            # attention for q-chunks 0/1 only touches the first seq half of
            # Q/K/V, so it is emitted right after half 0's projections and
            # overlaps half 1's loads/projections on the PE.
            with tc.tile_pool(name="ht", bufs=1) as htp, \
                 tc.tile_pool(name="wts", bufs=1) as wtp, \
                 tc.tile_pool(name="trig", bufs=1) as trig, \
                 tc.tile_pool(name="p1tmp", bufs=1) as tmp, \
                 tc.tile_pool(name="p2", bufs=1) as p2, \
                 tc.tile_pool(name="psr", bufs=1, space="PSUM") as psr, \
                 tc.tile_pool(name="psqkv", bufs=2, space="PSUM") as psqkv, \
                 tc.tile_pool(name="psvt", bufs=1, space="PSUM") as psvt, \
                 tc.tile_pool(name="pss", bufs=2, space="PSUM") as pss, \
                 tc.tile_pool(name="psav", bufs=1, space="PSUM") as psav, \
                 tc.tile_pool(name="psden", bufs=1, space="PSUM") as psden:

                cosr = trig.tile([128, SEQ], f32, name="cosr")
                nc.sync.dma_start(cosr[:], ins["cosb"])
                sinr = trig.tile([128, SEQ], f32, name="sinr")
                nc.sync.dma_start(sinr[:], ins["sinb_pre"])
                Rr = trig.tile([128, SEQ], f32, name="rr")

                wq_sb = wtp.tile([128, KT, DQ], bf16, name="wq_sb")
                wqT_ap = ins["wqT"].rearrange("(kt p) d -> p kt d", p=128)
                wkT_ap = ins["wkT"].rearrange("(kt p) d -> p kt d", p=128)
                wvT_ap = ins["wvT"].rearrange("(kt p) d -> p kt d", p=128)
                hT_ap = ins["hT"].rearrange("(kt p) s -> p kt s", p=128)

                def attn_part(qcs):
                    for h in range(QH):
                        for qc in qcs:
                            av = psav.tile([128, 512], f32, tag="av",
                                           name=f"av_{h}_{qc}")
                            den = psden.tile([128, 512], f32, tag="den",
                                             name=f"den_{h}_{qc}")
                            nkb = 4 * qc + 4
                            for kb in range(nkb):
                                ss = pss.tile([128, 512], f32, tag="s",
                                              name=f"ss_{h}_{qc}_{kb}")
                                nc.tensor.matmul(
                                    ss[:],
                                    lhsT=KTile[:, kb * 128:(kb + 1) * 128],
                                    rhs=QT[h][:, qc * 512:(qc + 1) * 512],
                                    start=True, stop=True)
                                es = p2.tile([128, 512], bf16, tag="es",
                                             bufs=18, name=f"es_{h}_{qc}_{kb}")
                                j = kb - 4 * qc
                                if j < 0:
                                    nc.scalar.activation(
                                        es[:], ss[:],
                                        mybir.ActivationFunctionType.Exp,
                                        bias=zero_f32[:], scale=SCALE)
                                else:
                                    if j > 0:
                                        nc.vector.memset(es[:, 0:j * 128], 0.0)
                                    nc.scalar.activation(
                                        es[:, j * 128:512], ss[:, j * 128:512],
                                        mybir.ActivationFunctionType.Exp,
                                        bias=zero_f32[:], scale=SCALE)
                                    nc.vector.tensor_mul(
                                        out=es[:, j * 128:(j + 1) * 128],
                                        in0=es[:, j * 128:(j + 1) * 128],
                                        in1=mask_sb[:])
                                nc.tensor.matmul(
                                    av[:], lhsT=Vn[:, kb, :], rhs=es[:],
                                    start=(kb == 0), stop=(kb == nkb - 1))
                                nc.tensor.matmul(
                                    den[:], lhsT=ones_sb[:], rhs=es[:],
                                    start=(kb == 0), stop=(kb == nkb - 1))
                            dinv = p2.tile([128, 512], f32, tag="dinv", bufs=2,
                                           name=f"dinv_{h}_{qc}")
                            nc.vector.reciprocal(dinv[:], den[:])
                            ao = p2.tile([128, 512], bf16, tag="ao", bufs=2,
                                         name=f"ao_{h}_{qc}")
                            nc.vector.tensor_mul(out=ao[:], in0=av[:],
                                                 in1=dinv[:])
                            nc.sync.dma_start(
                                ag_in[h][:, qc * 512:(qc + 1) * 512], ao[:])
                        if qcs[-1] == QC - 1:
                            # head complete: gather it across ranks while the
                            # next head's attention keeps the engines busy
                            nc.gpsimd.collective_compute(
                                "AllGather", mybir.AluOpType.bypass,
                                replica_groups=[list(range(NCORES))],
                                ins=[ag_in[h][:].opt()],
                                outs=[ag_out[h][:].opt()])

                for half in range(2):
                    h0 = half * HALF
                    hts = []
                    for kt in range(KT):
                        t = htp.tile([128, HALF], bf16, tag="ht", bufs=33,
                                     name=f"ht_{half}_{kt}")
                        nc.sync.dma_start(t[:], hT_ap[:, kt, h0:h0 + HALF])
                        hts.append(t)
                    if half == 0:
                        # weights after the first ht tiles so compute input
                        # wins the DMA queues
                        for kt in range(KT):
                            nc.sync.dma_start(wq_sb[:, kt, :], wqT_ap[:, kt, :])

                    # --- rms stats: sumsq over dim via ACT square +
                    # ones-matmul, one 512-chunk at a time (1 PSUM bank)
                    for c2 in range(HALF // 512):
                        c0 = h0 + c2 * 512
                        csl = slice(c2 * 512, (c2 + 1) * 512)
                        ps_r = psr.tile([128, 512], f32, tag="psr",
                                        name=f"ps_r_{half}_{c2}")
                        for kt in range(KT):
                            sq = tmp.tile([128, 512], bf16, tag="sq", bufs=3,
                                          name=f"sq_{half}_{c2}_{kt}")
                            nc.scalar.activation(
                                sq[:], hts[kt][:, csl],
                                mybir.ActivationFunctionType.Square,
                                bias=zero_bf16[:])
                            nc.tensor.matmul(
                                ps_r[:], lhsT=ones_sb[:], rhs=sq[:],
                                start=(kt == 0), stop=(kt == KT - 1))
                        srt = tmp.tile([128, 512], f32, tag="srt", bufs=2,
                                       name=f"srt_{half}_{c2}")
                        # sqrt(sumsq/DIM + EPS)
                        nc.scalar.activation(
                            srt[:], ps_r[:], mybir.ActivationFunctionType.Sqrt,
                            bias=eps_f32[:], scale=1.0 / DIM)
                        nc.vector.reciprocal(Rr[:, c0:c0 + 512], srt[:])
                        nc.vector.tensor_mul(
                            out=cosr[:, c0:c0 + 512], in0=cosr[:, c0:c0 + 512],
                            in1=Rr[:, c0:c0 + 512])
                        nc.vector.tensor_mul(
                            out=sinr[:, c0:c0 + 512], in0=sinr[:, c0:c0 + 512],
                            in1=Rr[:, c0:c0 + 512])

                    # --- QKV projections: blocks 0..3 = q heads, 4 = k, 5 = v
                    # K/V first so attention S-matmuls can be hoisted by the
                    # scheduler as soon as each q head completes
                    for b in (4, 5, 0, 1, 2, 3):
                        for ch in range(HALF // 512):
                            s0 = h0 + ch * 512
                            sl = slice(ch * 512, (ch + 1) * 512)
                            if b == 4:
                                wtile = [tmp.tile([128, 128], bf16, tag="wkv",
                                                  bufs=8, name=f"wk_{half}_{ch}_{kt}")
                                         for kt in range(KT)]
                                for kt in range(KT):
                                    nc.sync.dma_start(wtile[kt][:], wkT_ap[:, kt, :])
                            elif b == 5:
                                wtile = [tmp.tile([128, 128], bf16, tag="wkv",
                                                  bufs=8, name=f"wv_{half}_{ch}_{kt}")
                                         for kt in range(KT)]
                                for kt in range(KT):
                                    nc.sync.dma_start(wtile[kt][:], wvT_ap[:, kt, :])
                            ps = psqkv.tile([128, 512], f32, tag="qkv",
                                            name=f"ps_{half}_{b}_{ch}")
                            for kt in range(KT):
                                if b < 4:
                                    lhsT = wq_sb[:, kt, b * 128:(b + 1) * 128]
                                else:
                                    lhsT = wtile[kt][:]
                                nc.tensor.matmul(
                                    ps[:], lhsT=lhsT, rhs=hts[kt][:, sl],
                                    start=(kt == 0), stop=(kt == KT - 1))
                            if b == 5:
                                # V: scale by r, transpose 128-blocks to natural
                                vsc = tmp.tile([128, 512], bf16, tag="vsc",
                                               bufs=2, name=f"vsc_{half}_{ch}")
                                nc.vector.tensor_mul(
                                    out=vsc[:], in0=ps[:],
                                    in1=Rr[:, s0:s0 + 512])
                                for j in range(4):
                                    pvt = psvt.tile([128, 128], bf16, tag="vt",
                                                    name=f"pvt_{half}_{ch}_{j}")
                                    nc.tensor.transpose(
                                        pvt[:], vsc[:, j * 128:(j + 1) * 128],
                                        ident_sb[:])
                                    nc.vector.tensor_copy(
                                        out=Vn[:, s0 // 128 + j, :], in_=pvt[:])
                            else:
                                # rope: out = x*cosr + swap(x*sinr_pre)
                                dst = QT[b] if b < QH else KTile
                                m1 = tmp.tile([128, 512], f32, tag="m1", bufs=2,
                                              name=f"m1_{half}_{b}_{ch}")
                                nc.vector.tensor_mul(
                                    out=m1[:], in0=ps[:], in1=cosr[:, s0:s0 + 512])
                                m2p = tmp.tile([128, 512], f32, tag="m2p", bufs=2,
                                               name=f"m2p_{half}_{b}_{ch}")
                                nc.vector.tensor_mul(
                                    out=m2p[:], in0=ps[:], in1=sinr[:, s0:s0 + 512])
                                m2 = tmp.tile([128, 512], f32, tag="m2", bufs=2,
                                              name=f"m2_{half}_{b}_{ch}")
                                nc.sync.dma_start(m2[0:64, :], m2p[64:128, :])
                                nc.sync.dma_start(m2[64:128, :], m2p[0:64, :])
                                nc.vector.tensor_add(
                                    out=dst[:, s0:s0 + 512], in0=m1[:], in1=m2[:])

                    # attention over the q-chunks whose keys are now complete
                    attn_part(list(range(2 * half, 2 * half + 2)))

        # ---------------- phase 3: output projection ----------------
        # i-tiles grouped by gather (8 tiles per per-head AllGather); the wo
        # accumulation walks groups in gather-completion order so the PE only
        # waits on the last AllGather, and only if it has run out of work.
        with tc.tile_pool(name="attn", bufs=1) as ap_, \
             tc.tile_pool(name="wop", bufs=1) as wop, \
             tc.tile_pool(name="otmp", bufs=1) as otmp, \
             tc.tile_pool(name="pso", bufs=8, space="PSUM") as pso:
            wo_sb = wop.tile([128, KT, DQ], bf16, name="wo_sb")
            woT_ap = ins["woT"].rearrange("(it p) j -> p it j", p=128)
            for it in range(KT):
                nc.sync.dma_start(wo_sb[:, it, :], woT_ap[:, it, :])
            atiles = []   # [g][it8]
            for g in range(QH):
                ag_ap = ag_out[g].rearrange("(it p) s -> p it s", p=128)
                gt = []
                for it8 in range(NCORES):
                    at = ap_.tile([128, SEQ], bf16, tag="attn", bufs=32,
                                  name=f"attn_{g}_{it8}")
                    nc.sync.dma_start(at[:], ag_ap[:, it8, :])
                    gt.append(at)
                atiles.append(gt)
            NPASS = 2
            PSB = SB // NPASS     # 8 s-blocks per pass
            for p_ in range(NPASS):
                pos = [pso.tile([128, DQ], f32, tag="o",
                                name=f"po_{p_}_{i}") for i in range(PSB)]
                for g in range(QH):
                    for it8 in range(NCORES):
                        for i in range(PSB):
                            sb = p_ * PSB + i
                            nc.tensor.matmul(
                                pos[i][:],
                                lhsT=atiles[g][it8][:, sb * 128:(sb + 1) * 128],
                                rhs=wo_sb[:, g * NCORES + it8, :],
                                start=(g == 0 and it8 == 0),
                                stop=(g == QH - 1 and it8 == NCORES - 1))
                for i in range(PSB):
                    sb = p_ * PSB + i
                    ot = otmp.tile([128, DQ], f32, tag="ot", bufs=4,
                                   name=f"ot_{sb}")
                    nc.scalar.copy(ot[:], pos[i][:])
                    nc.sync.dma_start(out_ap[sb * 128:(sb + 1) * 128, :],
                                      ot[:])


def build_program(reps=1):
    nc = bacc.Bacc("TRN2", target_bir_lowering=False, debug=False,
                   num_devices=NCORES)
    in_specs = {
        "hT": ([DIM, SEQ], bf16),
        "wqT": ([DIM, DQ], bf16),
        "wkT": ([DIM, HD], bf16),
        "wvT": ([DIM, HD], bf16),
        "woT": ([DIM, DQ], bf16),
        "cosb": ([128, SEQ], f32),
        "sinb_pre": ([128, SEQ], f32),
        "maskdiag": ([128, 128], bf16),
        # width encodes reps so differently-unrolled builds can't alias in
        # the jit/AOT compile cache (keys include input avals)
        "ones128": ([128, 128 + (reps - 1)], bf16),
        "ident": ([128, 128], bf16),
    }
    ins = {}
    for name, (shape, dt) in in_specs.items():
        ins[name] = nc.dram_tensor(name, shape, dt, kind="ExternalInput").ap()
    out_t = nc.dram_tensor("out", [SEQ, DQ], f32, kind="ExternalOutput")
    with tile.TileContext(nc) as tc:
        for _ in range(reps):
            build_body(nc, tc, ins, out_t.ap())
    nc.compile()
    return nc


def kernel(**inputs):
    global LAST_RESULT
    maps = host_prep(**inputs)
    nc = build_program()
    res = run_bass_kernel_spmd(nc, maps, core_ids=list(range(NCORES)))
    LAST_RESULT = res
    out = np.concatenate([res.results[c]["out"] for c in range(NCORES)], axis=1)
    return out.astype(np.float32)
